# revision 1
# baseline (speedup 1.0000x reference)
"""Trainium2 Bass kernel for the HardResetSSMBlock problem.

y = silu(x @ W1 + b1) @ W2 + b2, masked per frame, with a periodic decay
scale on frames where (t+1) % 10 == 0.

Strategy: data-parallel over 8 NeuronCores (2 batch rows each -> 32768
tokens per core). The host feeds x pre-transposed into [n_tiles, 128, 512]
feature-major tiles, so each 512-token tile runs:
  DMA in (contiguous) -> MM1 (W1 stationary, X^T moving) -> Silu(+b1) on
  ACT writing float32r -> MM2 with H^T chunks as the stationary operand so
  Y lands token-major -> fused (mask * decay) scale via broadcast
  tensor_mul on DVE -> DMA out.

The per-token scale s = mask * decay is precomputed on host (a cheap [B,S]
elementwise product) and fed pre-transposed as [128, n_chunks] so it can
be applied as per-partition scalars on the token-major output.
"""

import numpy as np

B, S, D = 16, 16384, 128
RESET_PERIOD = 10
DECAY_FACTOR = 0.1
N_CORES = 8
TOK_PER_CORE = B * S // N_CORES  # 32768
TILE_TOK = 2048
CH = TILE_TOK // 128  # 4 chunks of 128 tokens
N_TILES = TOK_PER_CORE // TILE_TOK  # 64

# float32r matmuls stream 1 col/cycle on HW (vs 4 for fp32). f32r is a
# distinct rounding: a compute engine must produce the operand. MM2's
# stationary+moving come from on-chip ops, so it can use f32r cheaply
# (silu writes f32r; W2 converted once). MM1 reads x straight from DMA:
# if MM1_DMA_F32R, the DMA destination is declared f32r (numerics verified
# against fp32 on HW); otherwise MM1 runs plain fp32.
MM1_DMA_F32R = True
MM2_F32R = True
# Silu on hardware; CoreSim lacks it, so sim tests may override (e.g. Sigmoid)
ACT_FUNC = "Silu"

_CACHE = {}


def _build_nc():
    import concourse.bacc as bacc
    import concourse.tile as tile
    from concourse import mybir
    from concourse.bass import AP

    f32 = mybir.dt.float32
    f32r = mybir.dt.float32r

    nc = bacc.Bacc()
    xt_d = nc.dram_tensor(
        "x_t", [N_TILES, 128, TILE_TOK],
        f32r if MM1_DMA_F32R else f32, kind="ExternalInput"
    )
    st_d = nc.dram_tensor(
        "s_t", [128, TOK_PER_CORE // 128], f32, kind="ExternalInput"
    )
    w1_d = nc.dram_tensor("w1", [D, D], f32, kind="ExternalInput")
    w2_d = nc.dram_tensor("w2", [D, D], f32, kind="ExternalInput")
    b1_d = nc.dram_tensor("b1", [D, 1], f32, kind="ExternalInput")
    # partition-major output tiles: y_t[t, p, c, d] = y[(t*CH + c)*128 + p, d]
    # (host un-permutes; this makes the out-DMA write 2KB contiguous runs)
    y_d = nc.dram_tensor(
        "y_t", [N_TILES, 128, CH, D], f32, kind="ExternalOutput"
    )

    mm1_in_dt = f32r if MM1_DMA_F32R else f32
    mm2_dt = f32r if MM2_F32R else f32

    with tile.TileContext(nc) as tc:
        with (
            tc.tile_pool(name="const", bufs=1) as constp,
            tc.tile_pool(name="xt", bufs=6) as xtp,
            tc.tile_pool(name="ht", bufs=4) as htp,
            tc.tile_pool(name="yout", bufs=4) as youtp,
            tc.tile_pool(name="ps_ht", bufs=4, space="PSUM") as ps_htp,
            tc.tile_pool(name="ps_y", bufs=4, space="PSUM") as ps_yp,
        ):
            w1_raw = constp.tile([128, 128], f32)
            nc.gpsimd.dma_start(w1_raw[:], w1_d[:])
            if MM1_DMA_F32R:
                w1_s = constp.tile([128, 128], f32r, tag="w1r")
                nc.vector.tensor_copy(w1_s[:], w1_raw[:])
            else:
                w1_s = w1_raw
            w2_raw = constp.tile([128, 128], f32)
            nc.gpsimd.dma_start(w2_raw[:], w2_d[:])
            if MM2_F32R:
                w2_s = constp.tile([128, 128], f32r, tag="w2r")
                nc.vector.tensor_copy(w2_s[:], w2_raw[:])
            else:
                w2_s = w2_raw
            b1_s = constp.tile([128, 1], f32)
            nc.gpsimd.dma_start(b1_s[:], b1_d[:])
            st_s = constp.tile([128, TOK_PER_CORE // 128], f32)
            nc.gpsimd.dma_start(st_s[:], st_d[:])



            for t in range(N_TILES):
                s_xt = xtp.tile([128, TILE_TOK], mm1_in_dt)
                in_eng = nc.sync if t % 2 == 0 else nc.scalar
                in_eng.dma_start(s_xt[:], xt_d[t])

                s_ht = htp.tile([128, TILE_TOK], mm2_dt)
                s_y = youtp.tile([128, CH, 128], f32)
                for h in range(TILE_TOK // 512):
                    hs = slice(h * 512, (h + 1) * 512)
                    p_ht = ps_htp.tile([128, 512], f32)
                    nc.tensor.matmul(
                        p_ht[:], w1_s[:], s_xt[:, hs], start=True, stop=True
                    )
                    nc.scalar.activation(
                        s_ht[:, hs], p_ht[:],
                        getattr(mybir.ActivationFunctionType, ACT_FUNC),
                        bias=b1_s[:], scale=1.0,
                    )
                    p_y = ps_yp.tile([128, 4, 128], f32)
                    for c in range(4):
                        cc = h * 4 + c
                        nc.tensor.matmul(
                            p_y[:, c, :],
                            s_ht[:, cc * 128:(cc + 1) * 128], w2_s,
                            start=True, stop=True,
                        )
                    s_slice = st_s[:, t * CH + h * 4:t * CH + h * 4 + 4]
                    s_bcast = AP(
                        tensor=s_slice.tensor,
                        offset=s_slice.offset,
                        ap=list(s_slice.ap) + [[0, 128]],
                    )  # [128, 4, 128] with stride-0 feature dim
                    nc.vector.tensor_mul(
                        s_y[:, h * 4:(h + 1) * 4, :], p_y[:], s_bcast
                    )

                out_eng = nc.scalar if t % 2 == 0 else nc.sync
                out_eng.dma_start(y_d[t], s_y[:])

    nc.finalize()
    return nc


def _get_nc():
    if "nc" not in _CACHE:
        _CACHE["nc"] = _build_nc()
    return _CACHE["nc"]


def _host_prep(x, mask, W1, b1, W2, b2):
    """Shard inputs across 8 cores; pre-transpose x; per-token scale."""
    x = np.asarray(x, dtype=np.float32)
    mask = np.asarray(mask)
    W1 = np.ascontiguousarray(np.asarray(W1, dtype=np.float32))
    W2 = np.ascontiguousarray(np.asarray(W2, dtype=np.float32))
    b1 = np.asarray(b1, dtype=np.float32).reshape(D, 1)

    t = np.arange(S)
    decay = np.where((t + 1) % RESET_PERIOD == 0, DECAY_FACTOR, 1.0).astype(
        np.float32
    )
    s = mask.astype(np.float32) * decay[None, :]  # [B, S]

    # [B*S, D] -> per-core [N_TILES, D, TILE_TOK] feature-major tiles
    x_t_all = np.ascontiguousarray(
        x.reshape(N_CORES, N_TILES, TILE_TOK, D).transpose(0, 1, 3, 2)
    )

    rows_per_core = B // N_CORES
    in_maps = []
    for c in range(N_CORES):
        ss = s[c * rows_per_core:(c + 1) * rows_per_core].reshape(TOK_PER_CORE)
        s_t = np.ascontiguousarray(
            ss.reshape(TOK_PER_CORE // 128, 128).T
        )  # [128, n_chunks]
        in_maps.append(
            {
                "x_t": x_t_all[c],
                "s_t": s_t,
                "w1": W1,
                "w2": W2,
                "b1": b1,
            }
        )
    return in_maps


def kernel(x, mask, W1, b1, W2, b2, _trace=False):
    from concourse.bass_utils import run_bass_kernel_spmd

    b2 = np.asarray(b2, dtype=np.float32)

    nc = _get_nc()
    in_maps = _host_prep(x, mask, W1, b1, W2, b2)
    res = run_bass_kernel_spmd(
        nc, in_maps, list(range(N_CORES)), trace=_trace
    )
    if _trace:
        _CACHE["last_results"] = res
    # y_t[t, p, c, d] -> y[(t*CH + c)*128 + p, d]
    out = np.stack([res.results[c]["y_t"] for c in range(N_CORES)])
    out = np.ascontiguousarray(out.transpose(0, 1, 3, 2, 4)).reshape(B, S, D)
    if np.any(b2):
        # device computes (h @ W2) * s; the masked/decayed bias is added here
        t = np.arange(S)
        decay = np.where(
            (t + 1) % RESET_PERIOD == 0, DECAY_FACTOR, 1.0
        ).astype(np.float32)
        s = np.asarray(mask).astype(np.float32) * decay[None, :]
        out = out + s[:, :, None] * b2[None, None, :]
    return out



# revision 7
# speedup vs baseline: 1.6149x; 1.6149x over previous
"""Trainium2 Bass kernel for the HardResetSSMBlock problem.

y = silu(x @ W1 + b1) @ W2 + b2, masked per frame, with a periodic decay
scale on frames where (t+1) % 10 == 0.

Strategy: data-parallel over 8 NeuronCores (2 batch rows each -> 32768
tokens per core). Memory-bound problem, so all HBM traffic is bf16:
x is cast+transposed on host into [n_tiles, 128, 4096] feature-major
bf16 tiles, W1/W2 are cast to bf16, and y is written back as bf16
(host upcasts). PSUM accumulation stays f32, so the only precision
loss is bf16 operand/output quantization (~3e-3 rel).

Per 4096-token tile, in groups of 1024 tokens:
  MM1 (W1 stationary, X moving, 2x N=512) -> Silu(+b1) on ACT reading
  a 2-bank [128,1024] PSUM group, writing bf16 -> MM2 with 8 x 128-token
  H^T chunks stationary (FWL, bf16) so Y lands token-major -> fused
  (mask * decay) scale via per-partition broadcast tensor_mul on DVE,
  f32 PSUM -> bf16 SBUF -> 1MB DMA out per tile.

The per-token scale s = mask * decay is precomputed on host (cheap
[B,S] elementwise product) and fed pre-transposed as [128, n_chunks]
f32 so it can be applied as per-partition scalars on the token-major
output.
"""

import numpy as np

B, S, D = 16, 16384, 128
RESET_PERIOD = 10
DECAY_FACTOR = 0.1
N_CORES = 8
TOK_PER_CORE = B * S // N_CORES  # 32768
TILE_TOK = 4096
CH = TILE_TOK // 128  # 32 chunks of 128 tokens
N_TILES = TOK_PER_CORE // TILE_TOK  # 8
GRP = 1024  # tokens per PSUM group (2 banks)
G_PER_TILE = TILE_TOK // GRP  # 4
N_GRP = TOK_PER_CORE // GRP  # 32
PREFETCH = 3

# Silu on hardware; CoreSim lacks it, so sim tests may override (e.g. Sigmoid)
ACT_FUNC = "Silu"

_CACHE = {}


def _build_nc():
    import concourse.bacc as bacc
    import concourse.tile as tile
    from concourse import mybir
    from concourse.bass import AP

    f32 = mybir.dt.float32
    bf16 = mybir.dt.bfloat16

    nc = bacc.Bacc()
    xt_d = nc.dram_tensor(
        "x_t", [N_TILES, 128, TILE_TOK], bf16, kind="ExternalInput"
    )
    st_d = nc.dram_tensor(
        "s_t", [128, TOK_PER_CORE // 128], f32, kind="ExternalInput"
    )
    w1_d = nc.dram_tensor("w1", [D, D], bf16, kind="ExternalInput")
    w2_d = nc.dram_tensor("w2", [D, D], bf16, kind="ExternalInput")
    b1_d = nc.dram_tensor("b1", [D, 1], f32, kind="ExternalInput")
    # partition-major output tiles: y_t[t, p, c, d] = y[(t*CH + c)*128 + p, d]
    # (host un-permutes; this makes the out-DMA write 8KB contiguous runs)
    y_d = nc.dram_tensor(
        "y_t", [N_TILES, 128, CH, D], bf16, kind="ExternalOutput"
    )

    with tile.TileContext(nc) as tc:
        with (
            tc.tile_pool(name="const", bufs=1) as constp,
            tc.tile_pool(name="xt", bufs=PREFETCH + 1) as xtp,
            tc.tile_pool(name="ht", bufs=3) as htp,
            tc.tile_pool(name="yout", bufs=2) as youtp,
            tc.tile_pool(name="ps_ht", bufs=2, space="PSUM") as ps_htp,
            tc.tile_pool(name="ps_y", bufs=2, space="PSUM") as ps_yp,
        ):
            w1_s = constp.tile([128, 128], bf16)
            nc.sync.dma_start(w1_s[:], w1_d[:])
            b1_s = constp.tile([128, 1], f32)
            nc.sync.dma_start(b1_s[:], b1_d[:])
            w2_s = constp.tile([128, 128], bf16)
            nc.scalar.dma_start(w2_s[:], w2_d[:])
            st_s = constp.tile([128, TOK_PER_CORE // 128], f32)
            nc.scalar.dma_start(st_s[:], st_d[:])

            x_tiles = [None] * N_TILES
            y_tiles = [None] * N_TILES
            h_grp = [None] * N_GRP

            def in_dma(t):
                if t >= N_TILES:
                    return
                x_tiles[t] = xtp.tile([128, TILE_TOK], bf16, name="s_xt")
                eng = nc.sync if t % 2 == 0 else nc.scalar
                eng.dma_start(x_tiles[t][:], xt_d[t])

            for t in range(PREFETCH):
                in_dma(t)

            # software-pipelined by one group: PE order is
            # MM1(0), MM1(1), MM2(0), MM1(2), MM2(1), ... MM2(31)
            for g in range(N_GRP + 1):
                if g < N_GRP:
                    t = g // G_PER_TILE
                    if g % G_PER_TILE == 0:
                        in_dma(t + PREFETCH)
                        y_tiles[t] = youtp.tile(
                            [128, CH, D], bf16, name="s_y"
                        )
                    s_xt = x_tiles[t]
                    off = (g % G_PER_TILE) * GRP
                    ps = ps_htp.tile([128, GRP], f32)
                    for h in range(GRP // 512):
                        hs = slice(off + h * 512, off + (h + 1) * 512)
                        nc.tensor.matmul(
                            ps[:, h * 512:(h + 1) * 512], w1_s[:],
                            s_xt[:, hs], start=True, stop=True,
                        )
                    h_grp[g] = htp.tile([128, GRP], bf16, name="s_h")
                    nc.scalar.activation(
                        h_grp[g][:], ps[:],
                        getattr(mybir.ActivationFunctionType, ACT_FUNC),
                        bias=b1_s[:], scale=1.0,
                    )

                if g >= 1:
                    gp = g - 1
                    tp = gp // G_PER_TILE
                    c0 = (gp % G_PER_TILE) * (GRP // 128)  # chunk offset in tile
                    p_y = ps_yp.tile([128, GRP // 128, 128], f32)
                    for c in range(GRP // 128):
                        nc.tensor.matmul(
                            p_y[:, c, :],
                            h_grp[gp][:, c * 128:(c + 1) * 128], w2_s,
                            start=True, stop=True,
                        )
                    s_slice = st_s[:, tp * CH + c0:tp * CH + c0 + GRP // 128]
                    s_bcast = AP(
                        tensor=s_slice.tensor,
                        offset=s_slice.offset,
                        ap=list(s_slice.ap) + [[0, 128]],
                    )  # [128, GRP//128, 128] with stride-0 feature dim
                    nc.vector.tensor_mul(
                        y_tiles[tp][:, c0:c0 + GRP // 128, :], p_y[:], s_bcast
                    )
                    if gp % G_PER_TILE == G_PER_TILE - 1:
                        out_eng = nc.scalar if tp % 2 == 0 else nc.sync
                        out_eng.dma_start(y_d[tp], y_tiles[tp][:])

    nc.finalize()
    return nc


def _get_nc():
    if "nc" not in _CACHE:
        _CACHE["nc"] = _build_nc()
    return _CACHE["nc"]


def _host_prep(x, mask, W1, b1, W2, b2):
    """Shard inputs across 8 cores; pre-transpose x to bf16; scale vec."""
    from ml_dtypes import bfloat16

    x = np.asarray(x, dtype=np.float32)
    mask = np.asarray(mask)
    W1 = np.ascontiguousarray(np.asarray(W1, dtype=np.float32)).astype(bfloat16)
    W2 = np.ascontiguousarray(np.asarray(W2, dtype=np.float32)).astype(bfloat16)
    b1 = np.asarray(b1, dtype=np.float32).reshape(D, 1)

    t = np.arange(S)
    decay = np.where((t + 1) % RESET_PERIOD == 0, DECAY_FACTOR, 1.0).astype(
        np.float32
    )
    s = mask.astype(np.float32) * decay[None, :]  # [B, S]

    # [B*S, D] bf16 -> per-core [N_TILES, D, TILE_TOK] feature-major tiles
    x_bf = x.reshape(N_CORES, N_TILES, TILE_TOK, D).astype(bfloat16)
    x_t_all = np.ascontiguousarray(x_bf.transpose(0, 1, 3, 2))

    rows_per_core = B // N_CORES
    in_maps = []
    for c in range(N_CORES):
        ss = s[c * rows_per_core:(c + 1) * rows_per_core].reshape(TOK_PER_CORE)
        s_t = np.ascontiguousarray(
            ss.reshape(TOK_PER_CORE // 128, 128).T
        )  # [128, n_chunks]
        in_maps.append(
            {
                "x_t": x_t_all[c],
                "s_t": s_t,
                "w1": W1,
                "w2": W2,
                "b1": b1,
            }
        )
    return in_maps


def kernel(x, mask, W1, b1, W2, b2, _trace=False):
    from concourse.bass_utils import run_bass_kernel_spmd

    b2 = np.asarray(b2, dtype=np.float32)

    nc = _get_nc()
    in_maps = _host_prep(x, mask, W1, b1, W2, b2)
    res = run_bass_kernel_spmd(
        nc, in_maps, list(range(N_CORES)), trace=_trace
    )
    if _trace:
        _CACHE["last_results"] = res
    # y_t[t, p, c, d] -> y[(t*CH + c)*128 + p, d]
    out = np.stack([np.asarray(res.results[c]["y_t"]) for c in range(N_CORES)])
    out = out.transpose(0, 1, 3, 2, 4).astype(np.float32).reshape(B, S, D)
    out = np.ascontiguousarray(out)
    if np.any(b2):
        # device computes (h @ W2) * s; the masked/decayed bias is added here
        t = np.arange(S)
        decay = np.where(
            (t + 1) % RESET_PERIOD == 0, DECAY_FACTOR, 1.0
        ).astype(np.float32)
        s = np.asarray(mask).astype(np.float32) * decay[None, :]
        out = out + s[:, :, None] * b2[None, None, :]
    return out


# revision 8
# speedup vs baseline: 2.3236x; 1.4389x over previous
"""Trainium2 Bass kernel for the HardResetSSMBlock problem.

y = silu(x @ W1 + b1) @ W2 + b2, masked per frame, with a periodic decay
scale on frames where (t+1) % 10 == 0.

Strategy: the mask zeroes ~half the output tokens, and the op is
stateless per token, so the host packs only the unmasked tokens into a
dense stream (pure data movement -- all FLOPs stay on device), splits
it evenly across 8 NeuronCores, and scatters the device results back
into a zero-filled output. This halves both HBM traffic and compute.
All device HBM traffic is bf16 (PSUM accumulation stays f32; ~4e-3
max rel err). The remaining per-token scale fed to the device is the
decay factor at each kept token's original position.

Per 2048-token tile, in groups of 1024 tokens:
  MM1 (W1 stationary, X moving, 2x N=512) -> Silu(+b1) on ACT reading
  a 2-bank [128,1024] PSUM group, writing bf16 -> MM2 with 8 x 128-token
  H^T chunks stationary (FWL, bf16) so Y lands token-major -> decay
  scale via per-partition broadcast tensor_mul on DVE, f32 PSUM ->
  bf16 SBUF -> 512KB DMA out per tile.

The device kernel is compiled for ceil(max_core_tokens/2048) tiles and
cached per tile count, so any mask density (including all-ones) works.
"""

import numpy as np

B, S, D = 16, 16384, 128
RESET_PERIOD = 10
DECAY_FACTOR = 0.1
N_CORES = 8
TILE_TOK = 2048
CH = TILE_TOK // 128  # 16 chunks of 128 tokens
GRP = 1024  # tokens per PSUM group (2 banks)
G_PER_TILE = TILE_TOK // GRP  # 2
PREFETCH = 4

# Silu on hardware; CoreSim lacks it, so sim tests may override (e.g. Sigmoid)
ACT_FUNC = "Silu"

_CACHE = {}


def _build_nc(n_tiles):
    import concourse.bacc as bacc
    import concourse.tile as tile
    from concourse import mybir
    from concourse.bass import AP

    f32 = mybir.dt.float32
    bf16 = mybir.dt.bfloat16
    n_grp = n_tiles * G_PER_TILE
    n_chunks = n_tiles * CH

    nc = bacc.Bacc()
    xt_d = nc.dram_tensor(
        "x_t", [n_tiles, 128, TILE_TOK], bf16, kind="ExternalInput"
    )
    st_d = nc.dram_tensor("s_t", [128, n_chunks], f32, kind="ExternalInput")
    w1_d = nc.dram_tensor("w1", [D, D], bf16, kind="ExternalInput")
    w2_d = nc.dram_tensor("w2", [D, D], bf16, kind="ExternalInput")
    b1_d = nc.dram_tensor("b1", [D, 1], f32, kind="ExternalInput")
    # partition-major output tiles: y_t[t, p, c, d] = y[(t*CH + c)*128 + p, d]
    # (host un-permutes; this makes the out-DMA write 4KB contiguous runs)
    y_d = nc.dram_tensor(
        "y_t", [n_tiles, 128, CH, D], bf16, kind="ExternalOutput"
    )

    with tile.TileContext(nc) as tc:
        with (
            tc.tile_pool(name="const", bufs=1) as constp,
            tc.tile_pool(name="xt", bufs=PREFETCH + 1) as xtp,
            tc.tile_pool(name="ht", bufs=3) as htp,
            tc.tile_pool(name="yout", bufs=2) as youtp,
            tc.tile_pool(name="ps_ht", bufs=2, space="PSUM") as ps_htp,
            tc.tile_pool(name="ps_y", bufs=2, space="PSUM") as ps_yp,
        ):
            w1_s = constp.tile([128, 128], bf16)
            nc.sync.dma_start(w1_s[:], w1_d[:])
            b1_s = constp.tile([128, 1], f32)
            nc.sync.dma_start(b1_s[:], b1_d[:])
            w2_s = constp.tile([128, 128], bf16)
            nc.scalar.dma_start(w2_s[:], w2_d[:])
            st_s = constp.tile([128, n_chunks], f32)
            nc.scalar.dma_start(st_s[:], st_d[:])

            x_tiles = [None] * n_tiles
            y_tiles = [None] * n_tiles
            h_grp = [None] * n_grp

            def in_dma(t):
                if t >= n_tiles:
                    return
                x_tiles[t] = xtp.tile([128, TILE_TOK], bf16, name="s_xt")
                eng = nc.sync if t % 2 == 0 else nc.scalar
                eng.dma_start(x_tiles[t][:], xt_d[t])

            for t in range(PREFETCH):
                in_dma(t)

            # software-pipelined by one group: PE order is
            # MM1(0), MM1(1), MM2(0), MM1(2), MM2(1), ...
            for g in range(n_grp + 1):
                if g < n_grp:
                    t = g // G_PER_TILE
                    if g % G_PER_TILE == 0:
                        in_dma(t + PREFETCH)
                        y_tiles[t] = youtp.tile(
                            [128, CH, D], bf16, name="s_y"
                        )
                    s_xt = x_tiles[t]
                    off = (g % G_PER_TILE) * GRP
                    ps = ps_htp.tile([128, GRP], f32)
                    for h in range(GRP // 512):
                        hs = slice(off + h * 512, off + (h + 1) * 512)
                        nc.tensor.matmul(
                            ps[:, h * 512:(h + 1) * 512], w1_s[:],
                            s_xt[:, hs], start=True, stop=True,
                        )
                    h_grp[g] = htp.tile([128, GRP], bf16, name="s_h")
                    nc.scalar.activation(
                        h_grp[g][:], ps[:],
                        getattr(mybir.ActivationFunctionType, ACT_FUNC),
                        bias=b1_s[:], scale=1.0,
                    )

                if g >= 1:
                    gp = g - 1
                    tp = gp // G_PER_TILE
                    c0 = (gp % G_PER_TILE) * (GRP // 128)
                    p_y = ps_yp.tile([128, GRP // 128, 128], f32)
                    for c in range(GRP // 128):
                        nc.tensor.matmul(
                            p_y[:, c, :],
                            h_grp[gp][:, c * 128:(c + 1) * 128], w2_s,
                            start=True, stop=True,
                        )
                    s_slice = st_s[:, tp * CH + c0:tp * CH + c0 + GRP // 128]
                    s_bcast = AP(
                        tensor=s_slice.tensor,
                        offset=s_slice.offset,
                        ap=list(s_slice.ap) + [[0, 128]],
                    )  # [128, GRP//128, 128] with stride-0 feature dim
                    nc.vector.tensor_mul(
                        y_tiles[tp][:, c0:c0 + GRP // 128, :], p_y[:], s_bcast
                    )
                    if gp % G_PER_TILE == G_PER_TILE - 1:
                        out_eng = nc.scalar if tp % 2 == 0 else nc.sync
                        out_eng.dma_start(y_d[tp], y_tiles[tp][:])

    nc.finalize()
    return nc


def _get_nc(n_tiles):
    key = ("nc", n_tiles)
    if key not in _CACHE:
        _CACHE[key] = _build_nc(n_tiles)
    return _CACHE[key]


def kernel(x, mask, W1, b1, W2, b2, _trace=False):
    from ml_dtypes import bfloat16
    from concourse.bass_utils import run_bass_kernel_spmd

    x = np.asarray(x, dtype=np.float32)
    mask = np.asarray(mask)
    W1b = np.ascontiguousarray(np.asarray(W1, dtype=np.float32)).astype(
        bfloat16
    )
    W2b = np.ascontiguousarray(np.asarray(W2, dtype=np.float32)).astype(
        bfloat16
    )
    b1v = np.asarray(b1, dtype=np.float32).reshape(D, 1)
    b2 = np.asarray(b2, dtype=np.float32)

    t = np.arange(S)
    decay = np.where((t + 1) % RESET_PERIOD == 0, DECAY_FACTOR, 1.0).astype(
        np.float32
    )

    # pack unmasked tokens into a dense stream, split evenly over cores
    mask_flat = mask.reshape(-1)
    idx = np.flatnonzero(mask_flat)
    K = idx.size
    out_flat = np.zeros((B * S, D), dtype=np.float32)
    if K:
        k8 = -(-K // N_CORES)
        n_tiles = max(1, -(-k8 // TILE_TOK))
        cap = n_tiles * TILE_TOK
        tot = cap * N_CORES

        xp = np.zeros((tot, D), dtype=bfloat16)
        xp[:K] = x.reshape(B * S, D)[idx]
        sp = np.zeros(tot, dtype=np.float32)
        sp[:K] = np.broadcast_to(decay[None, :], (B, S)).reshape(-1)[idx]

        # feature-major tiles: [core, n_tiles, 128(d), TILE_TOK]
        x_t_all = np.ascontiguousarray(
            xp.reshape(N_CORES, n_tiles, TILE_TOK, D).transpose(0, 1, 3, 2)
        )
        s_all = sp.reshape(N_CORES, cap // 128, 128)

        in_maps = []
        for c in range(N_CORES):
            s_t = np.ascontiguousarray(s_all[c].T)  # [128, n_chunks]
            in_maps.append(
                {
                    "x_t": x_t_all[c],
                    "s_t": s_t,
                    "w1": W1b,
                    "w2": W2b,
                    "b1": b1v,
                }
            )

        nc = _get_nc(n_tiles)
        res = run_bass_kernel_spmd(
            nc, in_maps, list(range(N_CORES)), trace=_trace
        )
        if _trace:
            _CACHE["last_results"] = res
        # y_t[t, p, c, d] -> packed token (t*CH + c)*128 + p
        yp = np.stack(
            [np.asarray(res.results[c]["y_t"]) for c in range(N_CORES)]
        )
        yp = (
            yp.transpose(0, 1, 3, 2, 4)
            .astype(np.float32)
            .reshape(N_CORES * cap, D)
        )
        out_flat[idx] = yp[:K]

    out = out_flat.reshape(B, S, D)
    if np.any(b2):
        # device computes (h @ W2) * s; the masked/decayed bias is added here
        s = mask.astype(np.float32) * decay[None, :]
        out = out + s[:, :, None] * b2[None, None, :]
    return out
